# revision 2
# baseline (speedup 1.0000x reference)
"""Trainium2 Bass kernel: cosine-similarity KNN -> COO sparse assembly.

Single-core design (wall-clock optimized: the axon host->device link at
~50 MB/s dominates, so minimize wire bytes and host-side build/compile time
rather than distributing across cores):
  host: l2-normalize rows (fp32), split each element into bf16 hi + bf16 mid
        (x = hi + mid, ~17 significant bits), transpose to [C, N] planes;
        y zero-padded to NYP = 53248 cols. Transfers issued async so the
        Bass build + walrus compile overlap the wire time.
  device (one NeuronCore, For_i hardware loop over 192 block-pairs):
     stream y in [128, 4096] bf16 tiles (4 regions of 1024 per tile);
     per region and 128-row block: 8 matmuls (xh/xm x yh/ym over 2
     512-chunks) accumulate exact-ish fp32 sims in PSUM; DVE max8 +
     max_index directly on PSUM give per-region top-8 values + global col
     ids (index + region offset). After 52 regions: max8/match_replace/max8
     merge to the global top-10 (descending, ties -> lower col because
     reduce_min over matched ids), column lookup by value-matching
     (is_equal + mul + reduce_min), fp32 softmax of top-10/tau via ACT Exp.
     Outputs [NX,10] fp32 values + int32 cols (~3.9 MB fetched back).

Top-k exactness: a 1024-col region would need >= 9 of a row's global
top-16 to defeat per-region top-8 (P ~ 1e-11 per row). Values are fp32
accumulations of bf16-pair products (input rounding ~2^-17 relative),
matching the fp32 reference to ~1e-6.
"""

import numpy as np

NX, NY, C, K = 49152, 50000, 128, 10
TAU = 0.05
NYP = 53248                  # padded y columns: 13 tiles x 4096
REG = 1024                   # top-8 region width
YTW = 4096                   # y DMA tile width (4 regions)
BIG = 131072.0
PSUM_DIRECT = True

_CACHE = {}


def _build_nc():
    import concourse.bacc as bacc
    import concourse.mybir as mybir
    from concourse import tile
    import concourse.bass as bass

    f32 = mybir.dt.float32
    bf16 = mybir.dt.bfloat16
    i32 = mybir.dt.int32
    u16 = mybir.dt.uint16

    NITER = NX // 256
    NYT = NYP // YTW
    NREG = NYP // REG
    V8W = NREG * 8
    SC = float(1.0 / TAU)

    nc = bacc.Bacc("TRN2", target_bir_lowering=False, debug=False,
                   enable_asserts=False, num_devices=1)

    xh = nc.dram_tensor("xh", [C, NX], bf16, kind="ExternalInput")
    xm = nc.dram_tensor("xm", [C, NX], bf16, kind="ExternalInput")
    yh = nc.dram_tensor("yh", [C, NYP], bf16, kind="ExternalInput")
    ym = nc.dram_tensor("ym", [C, NYP], bf16, kind="ExternalInput")
    vals_o = nc.dram_tensor("vals_o", [NX, K], f32, kind="ExternalOutput")
    cols_o = nc.dram_tensor("cols_o", [NX, K], i32, kind="ExternalOutput")

    AX = mybir.AxisListType.X
    OP = mybir.AluOpType

    with tile.TileContext(nc) as tc:
        with (
            tc.tile_pool(name="xw", bufs=2) as xp,
            tc.tile_pool(name="yt", bufs=3) as yp,
            tc.tile_pool(name="ps", bufs=3, space="PSUM") as pp,
            tc.tile_pool(name="st", bufs=2) as sp,
            tc.tile_pool(name="mg", bufs=2) as mp,
        ):
            with tc.For_i(0, NITER, 1) as it:
                xt = {}
                for blk in (0, 1):
                    for nm, src in (("h", xh), ("m", xm)):
                        t = xp.tile([C, 128], bf16, tag=f"x{nm}{blk}",
                                    name=f"x{nm}{blk}")
                        nc.sync.dma_start(
                            out=t[:], in_=src[:, bass.ds(it * 256 + blk * 128, 128)])
                        xt[(nm, blk)] = t
                v8 = [sp.tile([128, V8W], f32, tag=f"v8{b}", name=f"v8{b}")
                      for b in (0, 1)]
                g8 = [sp.tile([128, V8W], f32, tag=f"g8{b}", name=f"g8{b}")
                      for b in (0, 1)]
                for yt_i in range(NYT):
                    ysl = slice(yt_i * YTW, (yt_i + 1) * YTW)
                    yht = yp.tile([C, YTW], bf16, tag="yh")
                    nc.sync.dma_start(out=yht[:], in_=yh[:, ysl])
                    ymt = yp.tile([C, YTW], bf16, tag="ym")
                    nc.sync.dma_start(out=ymt[:], in_=ym[:, ysl])
                    for blk in (0, 1):
                        for q in range(YTW // REG):
                            r = yt_i * (YTW // REG) + q
                            ps = pp.tile([128, REG], f32, tag="ps")
                            for wi, xw in enumerate((xt[("h", blk)], xt[("m", blk)])):
                                for c in (0, 1):
                                    ys = slice(q * REG + c * 512,
                                               q * REG + (c + 1) * 512)
                                    psl = ps[:, c * 512:(c + 1) * 512]
                                    nc.tensor.matmul(psl, xw[:], yht[:, ys],
                                                     start=(wi == 0), stop=False)
                                    nc.tensor.matmul(psl, xw[:], ymt[:, ys],
                                                     start=False, stop=(wi == 1))
                            r8 = slice(r * 8, (r + 1) * 8)
                            if PSUM_DIRECT:
                                sim_ap = ps[:]
                            else:
                                ssb = sp.tile([128, REG], f32, tag="ssb")
                                nc.scalar.copy(ssb[:], ps[:])
                                sim_ap = ssb[:]
                            nc.vector.max(v8[blk][:, r8], sim_ap)
                            i8t = mp.tile([128, 8], u16, tag="i8t")
                            nc.vector.max_index(i8t[:], v8[blk][:, r8], sim_ap)
                            nc.vector.tensor_scalar(
                                g8[blk][:, r8], i8t[:], float(r * REG), None, OP.add)
                for blk in (0, 1):
                    m1 = mp.tile([128, 8], f32, tag="m1")
                    nc.vector.max(m1[:], v8[blk][:])
                    vrep = sp.tile([128, V8W], f32, tag="vrep")
                    nc.vector.match_replace(vrep[:], m1[:], v8[blk][:], -3.0e38)
                    m2 = mp.tile([128, 8], f32, tag="m2")
                    nc.vector.max(m2[:], vrep[:])
                    v10 = mp.tile([128, K], f32, tag="v10")
                    nc.vector.tensor_copy(v10[:, 0:8], m1[:])
                    nc.vector.tensor_copy(v10[:, 8:K], m2[:, 0:2])
                    g8b = sp.tile([128, V8W], f32, tag="g8b")
                    nc.vector.tensor_scalar(g8b[:], g8[blk][:], BIG, None, OP.subtract)
                    colsf = mp.tile([128, K], f32, tag="colsf")
                    for k in range(K):
                        msk = sp.tile([128, V8W], f32, tag="msk")
                        nc.vector.tensor_scalar(
                            msk[:], v8[blk][:], v10[:, k:k + 1], None, OP.is_equal)
                        t2 = sp.tile([128, V8W], f32, tag="t2")
                        nc.vector.tensor_mul(t2[:], msk[:], g8b[:])
                        nc.vector.tensor_reduce(
                            colsf[:, k:k + 1], t2[:], AX, OP.min)
                    colsb = mp.tile([128, K], f32, tag="colsb")
                    nc.vector.tensor_scalar(colsb[:], colsf[:], BIG, None, OP.add)
                    colsi = mp.tile([128, K], i32, tag="colsi")
                    nc.vector.tensor_copy(colsi[:], colsb[:])
                    mx = mp.tile([128, 1], f32, tag="mx")
                    nc.vector.tensor_reduce(mx[:], v10[:], AX, OP.max)
                    d = mp.tile([128, K], f32, tag="d")
                    nc.vector.tensor_scalar(d[:], v10[:], mx[:, 0:1], None,
                                            OP.subtract)
                    e = mp.tile([128, K], f32, tag="e")
                    nc.scalar.activation(e[:], d[:],
                                         mybir.ActivationFunctionType.Exp,
                                         bias=0.0, scale=SC)
                    s = mp.tile([128, 1], f32, tag="s")
                    nc.vector.tensor_reduce(s[:], e[:], AX, OP.add)
                    rs = mp.tile([128, 1], f32, tag="rs")
                    nc.vector.reciprocal(rs[:], s[:])
                    vout = mp.tile([128, K], f32, tag="vout")
                    nc.vector.tensor_scalar(vout[:], e[:], rs[:, 0:1], None, OP.mult)
                    ro = bass.ds(it * 256 + blk * 128, 128)
                    nc.sync.dma_start(out=vals_o[ro, :], in_=vout[:])
                    nc.sync.dma_start(out=cols_o[ro, :], in_=colsi[:])
    nc.finalize()
    return nc


def _split_bf16_t(a, n_out):
    """Rows of a [N, C] fp32 -> l2-normalized, split to (hi, mid) bf16
    planes in transposed [C, n_out] layout (zero-padded)."""
    import ml_dtypes
    a = np.asarray(a, dtype=np.float32)
    n2 = np.einsum("ij,ij->i", a, a, dtype=np.float32)
    inv = (1.0 / np.maximum(np.sqrt(n2), 1e-12)).astype(np.float32)
    q = a * inv[:, None]
    xt = np.zeros((C, n_out), dtype=np.float32)
    xt[:, :a.shape[0]] = q.T
    u = xt.view(np.uint32)
    hb = ((u + np.uint32(0x7FFF) + ((u >> np.uint32(16)) & np.uint32(1)))
          >> np.uint32(16)).astype(np.uint16)
    hi = (hb.astype(np.uint32) << np.uint32(16)).view(np.float32)
    np.subtract(xt, hi, out=xt)
    um = xt.view(np.uint32)
    mb = ((um + np.uint32(0x7FFF) + ((um >> np.uint32(16)) & np.uint32(1)))
          >> np.uint32(16)).astype(np.uint16)
    return hb.view(ml_dtypes.bfloat16), mb.view(ml_dtypes.bfloat16)


def _get_compiled(nc):
    if "compiled" in _CACHE:
        return _CACHE["compiled"]
    import jax
    import ml_dtypes
    from concourse.bass2jax import (_bass_exec_p, partition_id_tensor,
                                    install_neuronx_cc_hook)
    install_neuronx_cc_hook()
    out_avals = (jax.core.ShapedArray((NX, K), np.float32),
                 jax.core.ShapedArray((NX, K), np.int32))
    in_names = ("xh", "xm", "yh", "ym", "vals_o", "cols_o", "partition_id")

    def _body(*args):
        operands = list(args) + [partition_id_tensor()]
        outs = _bass_exec_p.bind(
            *operands, out_avals=out_avals, in_names=in_names,
            out_names=("vals_o", "cols_o"), lowering_input_output_aliases=(),
            sim_require_finite=True, sim_require_nnan=True, nc=nc)
        return tuple(outs)

    jitted = jax.jit(_body, donate_argnums=(4, 5), keep_unused=True)
    compiled = jitted.lower(
        jax.ShapeDtypeStruct((C, NX), ml_dtypes.bfloat16),
        jax.ShapeDtypeStruct((C, NX), ml_dtypes.bfloat16),
        jax.ShapeDtypeStruct((C, NYP), ml_dtypes.bfloat16),
        jax.ShapeDtypeStruct((C, NYP), ml_dtypes.bfloat16),
        jax.ShapeDtypeStruct((NX, K), np.float32),
        jax.ShapeDtypeStruct((NX, K), np.int32),
    ).compile()
    _CACHE["compiled"] = compiled
    return compiled


def kernel(feat_x, feat_y):
    import jax
    dev = jax.devices()[0]

    # y first: its transfer (27 MB) overlaps x prep + build + compile
    yh, ym = _split_bf16_t(np.asarray(feat_y)[0], NYP)
    yh_d = jax.device_put(yh, dev)
    ym_d = jax.device_put(ym, dev)
    xh, xm = _split_bf16_t(np.asarray(feat_x)[0], NX)
    xh_d = jax.device_put(xh, dev)
    xm_d = jax.device_put(xm, dev)
    z1_d = jax.device_put(np.zeros((NX, K), np.float32), dev)
    z2_d = jax.device_put(np.zeros((NX, K), np.int32), dev)

    if "nc" not in _CACHE:
        _CACHE["nc"] = _build_nc()
    compiled = _get_compiled(_CACHE["nc"])

    vo, co = compiled(xh_d, xm_d, yh_d, ym_d, z1_d, z2_d)
    vals = np.asarray(vo, dtype=np.float32).reshape(-1)
    cols = np.clip(np.asarray(co, dtype=np.int64).reshape(-1), 0, NY - 1)
    rows = np.repeat(np.arange(NX, dtype=np.int32), K)
    return vals, rows, cols.astype(np.int32)
